# revision 47
# baseline (speedup 1.0000x reference)
"""Multi-head cross-attention TRN2 Bass kernel, sharded over 8 NeuronCores.

Problem (nn_MultiHeadCrossAttention): B=2, Sq=1024, Skv=4096 (text+image+
audio+video), hidden=1024, heads=16, head_dim=64, out=4096.

Sharding: core c = 4*b + hg handles batch b and head-group hg (4 heads).
Schedule — one software pipeline keeping the PE busy end to end:
  P1:  Q proj.  fft streams as 8 x 1MB chunks rotated over the three DMA
       queues (SP/Act/Pool); all inputs are HOST-PRE-TILED to [128, X]
       p-major layouts so each DMA lowers to ~128 large descriptors
       instead of thousands of sub-KB ones (descriptor processing, not
       HBM bytes, was the phase bound).  K proj + V proj for kv block 0
       run here too; kv block DMAs trickle via a 3-buffer pool so they
       don't steal fft bandwidth.
  C:   per head: scores -> exp(s/16) -> PV, software-pipelined one kv
       tile behind, ACT-paced.  Head 0's PE slack absorbs K proj
       (blocks 1-7), V proj (tiles 4-31) and the kv DMA trickle.
       Normalization of head h runs inside head h+1's window: the
       softmax reciprocal is exp(-ln(den)) on ACT (same activation
       table set as exp, no DVE 1-lane reciprocal on the critical
       path), expanded over partitions by a rank-1 PE matmul.  kt/qt
       half-duplication (K=128 score trick) is one batched SBUF DMA per
       head except the streaming head 0.  Phase-D partial for head-pair
       0 (32 output tiles) interleaves into heads 2-3.  bv folds into
       the host-side output bias (bo' = bo + Wo @ bv).
  D:   pair-1 partial as a tail on a 4-deep PSUM slot rotation with
       copies split DVE/ACT.  Data-dependent rank-1 fill matmuls keep
       the PE active through the final norm chain (an idle window would
       re-throttle the PE clock gate).  NOTE: the PE thermal/firmware
       throttle tends to kick in ~270us into the kernel, so the tail
       runs at ~1.2 GHz regardless — keeping it short matters more than
       keeping it dense.
Host sums the 8 per-(batch, pair) fp16 partials and adds bo'.
"""

import numpy as np

import bass_rust
import concourse.bass as bass
import concourse.mybir as mybir
import concourse.tile as tile
from concourse.bass_utils import run_bass_kernel_spmd
from concourse.vector_clock import ScopedClock

# ---------------------------------------------------------------------------
# Workarounds for walrus per-instruction sync-wait caps (this walrus build
# rejects instructions carrying more waits than the ISA slot count; Tile's
# sem assignment can attach more). Split excess waits onto single-wait nops.
# ---------------------------------------------------------------------------
import re as _re

_VC_RE = _re.compile(r"VectorClock\(\[([0-9, ]*)\]\)")


def _vc_values(vc):
    m = _VC_RE.match(repr(vc))
    assert m, repr(vc)
    s = m.group(1).strip()
    return [int(x) for x in s.split(",")] if s else []


def _split_excess_waits(tc, ordered_instructions_by_block, max_waits=1):
    nc = tc.nc
    for _bb, insts in ordered_instructions_by_block.items():
        out = []
        for inst in insts:
            si = inst.sync_info
            waits = list(si.on_wait) if si and si.on_wait else []
            if len(waits) > max_waits:
                keep = waits[:max_waits]
                for w in waits[max_waits:]:
                    nop = mybir.InstNoOp(
                        name=nc.get_next_instruction_name(), ins=[], outs=[]
                    )
                    nop.engine = inst.engine
                    nop.sync_info = bass_rust.SyncInfo(on_wait=[w], on_update=[])
                    nc.register_instruction(nop)
                    out.append(nop)
                inst.sync_info = bass_rust.SyncInfo(
                    on_wait=keep, on_update=list(si.on_update or [])
                )
            out.append(inst)
        insts[:] = out


_orig_lower = tile.TileContext._lower_ordered_insts


def _lower_with_split(self, postordered_blocks):
    _split_excess_waits(self, postordered_blocks)
    return _orig_lower(self, postordered_blocks)


def _drain_and_barrier_split(self, tick_clock, wait_clock):
    vals = _vc_values(tick_clock.global_clock)
    for proc_idx, tick in enumerate(vals):
        if tick <= 0:
            continue
        single = [0] * len(vals)
        single[proc_idx] = tick
        nop_inst = self.nc.sync.nop(nofuse=True, hint=f"drain_wait_p{proc_idx}")
        wait_clock.add_sem_waits(
            nop_inst.ins, ScopedClock({None: bass_rust.VectorClock(single)})
        )
    self.nc.sync.drain()
    self.nc.all_engine_barrier()
    assert self.sems is not None
    popped = self.nc._tile_sem_poison_stack.pop()
    assert popped is self._sem_poison
    self.nc.clear_and_free_semaphores(list(self.sems.allocated().values()))
    self.nc.all_engine_barrier()


tile.TileContext._lower_ordered_insts = _lower_with_split
tile.TileContext._drain_and_barrier = _drain_and_barrier_split

# ---------------------------------------------------------------------------
# Problem constants (hardcoded per contract)
# ---------------------------------------------------------------------------
B = 2
SQ = 1024
SKV = 4096
HID = 1024
HEADS = 16
DH = 64
DOUT = 4096
NCORES = 8
HG = 4  # head-groups (cores per batch)
GHEADS = HEADS // HG  # heads per group = 4
GF = GHEADS * DH  # feature slice per group = 256
NPAIR = GHEADS // 2  # head pairs per group = 2

F32 = mybir.dt.float32
BF16 = mybir.dt.bfloat16
FP16 = mybir.dt.float16
DT_MM = BF16
NP_MM = "bfloat16"
Exp = mybir.ActivationFunctionType.Exp
Ln = mybir.ActivationFunctionType.Ln
MUL = mybir.AluOpType.mult
ADD = mybir.AluOpType.add

NKVT = SKV // 128  # 32 kv tiles
NKVB = 8  # kv blocks (512 wide)
NFT_Q = 4096 // 128  # 32 contraction tiles for Q proj
NFT_KV = HID // 128  # 8 contraction tiles for K/V proj
NSQH = SQ // 512  # 2 sq halves
NJT = DOUT // 128  # 32 output row tiles

_NC_CACHE = {}


def build():
    if "nc" in _NC_CACHE:
        return _NC_CACHE["nc"]
    nc = bass.Bass()

    # All inputs are host-pre-tiled to [128, X] with per-partition-contiguous
    # layout: a DMA then lowers to a few large regular descriptors instead of
    # thousands of sub-KB ones (4D access patterns were saturating all 16 DMA
    # queues for ~50us).
    fft = nc.declare_dram_parameter("fft", [128, 8, 4, SQ], DT_MM, isOutput=False)
    kvt = nc.declare_dram_parameter("kvt", [128, NKVB, NFT_KV, 512], DT_MM,
                                    isOutput=False)
    wqt = nc.declare_dram_parameter("wqt", [128, NFT_Q, NPAIR, 128], DT_MM,
                                    isOutput=False)
    wkt = nc.declare_dram_parameter("wkt", [128, NFT_KV, NPAIR, 128], DT_MM,
                                    isOutput=False)
    wvt = nc.declare_dram_parameter("wvt", [128, NFT_KV, GF], DT_MM,
                                    isOutput=False)
    wot = nc.declare_dram_parameter("wot", [128, NPAIR, DOUT], DT_MM,
                                    isOutput=False)
    bq = nc.declare_dram_parameter("bq", [128, NPAIR], F32, isOutput=False)
    bk = nc.declare_dram_parameter("bk", [128, NPAIR], F32, isOutput=False)
    outp = nc.declare_dram_parameter("outp", [NPAIR, DOUT, SQ], FP16, isOutput=True)

    with tile.TileContext(nc) as tc:
        with (
            tc.tile_pool(name="hold", bufs=1) as hold,
            tc.tile_pool(name="misc", bufs=1) as misc,
            tc.tile_pool(name="kvs", bufs=3) as kvs,
        ):
            # ---- long-lived tiles ----
            wkt_r = hold.tile([128, NFT_KV, NPAIR, 128], DT_MM, tag="wkt")
            nc.sync.dma_start(out=wkt_r[:], in_=wkt[:])
            wvt_r = hold.tile([128, NFT_KV, GF], DT_MM, tag="wvt")
            wot_r = hold.tile([128, NPAIR, DOUT], DT_MM, tag="wot")
            wqt_r = hold.tile([128, NFT_Q, NPAIR, 128], DT_MM, tag="wqt")
            bq_t = misc.tile([128, NPAIR], F32, tag="bq")
            nc.sync.dma_start(out=bq_t[:], in_=bq[:])
            bk_t = misc.tile([128, NPAIR], F32, tag="bk")
            nc.sync.dma_start(out=bk_t[:], in_=bk[:])

            ones_f = misc.tile([128, GHEADS], F32, tag="ones_f")
            nc.vector.memset(ones_f[:], 1.0)
            ones_row = misc.tile([1, DH], DT_MM, tag="ones_row")
            nc.vector.tensor_copy(ones_row[:], ones_f[0:1, 0:1].broadcast_to([1, DH]))

            qt_r = hold.tile([128, GHEADS, SQ], DT_MM, tag="qt")
            kt_r = hold.tile([128, GHEADS, SKV], DT_MM, tag="kt")
            v_r = hold.tile([128, NKVT, GHEADS, 128], DT_MM, tag="v")
            att_r = hold.tile([128, NPAIR, SQ], DT_MM, tag="att")

            kv_blocks = [None] * NKVB

            def kv_dma(kb):
                kv_t = kvs.tile([128, NFT_KV, 512], DT_MM, tag="kv",
                                name=f"kv{kb}")
                kv_blocks[kb] = kv_t
                # two half-DMAs so K proj can start on ft 0-3 early
                nc.gpsimd.dma_start(out=kv_t[:, 0:4, :], in_=kvt[:, kb, 0:4, :])
                nc.gpsimd.dma_start(out=kv_t[:, 4:8, :], in_=kvt[:, kb, 4:8, :])

            def k_chunk(kb, pr, pool, tag, dup_eng=None):
                kv_t = kv_blocks[kb]
                kb_sl = slice(512 * kb, 512 * (kb + 1))
                kt_ps = pool.tile([128, 512], F32, tag=tag,
                                  name=f"kt_ps{kb}_{pr}")
                for ft in range(NFT_KV):
                    nc.tensor.matmul(
                        kt_ps[:],
                        wkt_r[:, ft, pr, :],
                        kv_t[:, ft, :],
                        start=(ft == 0),
                        stop=(ft == NFT_KV - 1),
                    )
                nc.vector.tensor_scalar(
                    kt_r[0:64, 2 * pr, kb_sl],
                    kt_ps[0:64, :],
                    bk_t[0:64, pr : pr + 1],
                    None,
                    ADD,
                )
                nc.vector.tensor_scalar(
                    kt_r[64:128, 2 * pr + 1, kb_sl],
                    kt_ps[64:128, :],
                    bk_t[64:128, pr : pr + 1],
                    None,
                    ADD,
                )
                if pr == 0:
                    dup = dup_eng or nc.gpsimd
                    dup.dma_start(
                        out=kt_r[64:128, 0, kb_sl],
                        in_=kt_r[0:64, 0, kb_sl],
                    )

            def v_chunk(kvt_i, pool, tag):
                """Project V for one 128-wide kv tile (one kl chunk)."""
                kb, kl = divmod(kvt_i, 4)
                kv_t = kv_blocks[kb]
                v_ps = pool.tile([128, GF], F32, tag=tag,
                                 name=f"v_ps{kvt_i}")
                for ft in range(NFT_KV):
                    nc.tensor.matmul(
                        v_ps[:],
                        kv_t[:, ft, 128 * kl : 128 * (kl + 1)],
                        wvt_r[:, ft, :],
                        start=(ft == 0),
                        stop=(ft == NFT_KV - 1),
                    )
                nc.vector.tensor_copy(
                    v_r[:, kvt_i, :, 0:DH],
                    v_ps.rearrange("p (h d) -> p h d", h=GHEADS),
                )
                nc.vector.tensor_copy(v_r[:, kvt_i, :, DH : DH + 1], ones_f[:, :])

            # ================= Phase 1: Q + K projections =================
            with (
                nc.named_scope("phaseAB_proj"),
                tc.tile_pool(name="ffts", bufs=5) as ffts,
                tc.tile_pool(name="psA", bufs=2, space="PSUM") as psA,
                tc.tile_pool(name="psB", bufs=4, space="PSUM") as psB,
            ):
                qt_ps = [
                    psA.tile([128, SQ], F32, tag="psA", name=f"qt_ps{i}")
                    for i in range(NPAIR)
                ]  # one [128, 1024] accumulator per pair (2 PSUM banks)
                fft_chunks = {}

                def fft_dma(c):
                    fft_t = ffts.tile([128, 4, SQ], DT_MM, tag="fft",
                                      name=f"fftc{c}")
                    eng = (nc.scalar, nc.sync, nc.gpsimd)[c % 3]
                    eng.dma_start(out=fft_t[:], in_=fft[:, c, :, :])
                    fft_chunks[c] = fft_t

                def a_step(kt):
                    c, e = divmod(kt, 4)
                    for pr in range(NPAIR):
                        for sh in range(NSQH):
                            sq = slice(512 * sh, 512 * (sh + 1))
                            nc.tensor.matmul(
                                qt_ps[pr][:, sq],
                                wqt_r[:, kt, pr, :],
                                fft_chunks[c][:, e, sq],
                                start=(kt == 0),
                                stop=(kt == NFT_Q - 1),
                            )

                kv_dma(0)
                nc.gpsimd.dma_start(out=wqt_r[:, 0:8], in_=wqt[:, 0:8])
                for c in range(5):
                    fft_dma(c)
                for c in range(1, 4):
                    nc.gpsimd.dma_start(out=wqt_r[:, 8 * c : 8 * (c + 1)],
                                        in_=wqt[:, 8 * c : 8 * (c + 1)])
                for kt in range(NFT_Q):
                    if kt % 4 == 2 and kt // 4 + 5 <= 7:
                        fft_dma(kt // 4 + 5)
                    a_step(kt)
                    if kt == 0:
                        k_chunk(0, 0, psB, "psB", nc.sync)
                        k_chunk(0, 1, psB, "psB", nc.sync)
                    if kt == 8:
                        nc.sync.dma_start(out=wvt_r[:], in_=wvt[:])
                    if kt >= 28:
                        v_chunk(kt - 28, psB, "psB")  # kv block 0
                kv_dma(1)  # prefetch for head 0's first K chunk

                # activation-table warm: force the natural_log_exp set to
                # load now (ACT idle; all phase-AB DMAs already issued).
                warm = misc.tile([1, 64], F32, tag="warm")
                nc.vector.memset(warm[:], 1.0)
                warm2 = misc.tile([1, 64], F32, tag="warm2")
                nc.scalar.activation(warm2[:], warm[:], Ln)
                nc.scalar.activation(warm2[:], warm2[:], Exp)

                Ident = mybir.ActivationFunctionType.Identity
                for pr in range(NPAIR):
                    # split the four bias-adds across DVE and ACT so the
                    # AB->C transition isn't serialized on one engine
                    nc.vector.tensor_scalar(
                        qt_r[0:64, 2 * pr, :],
                        qt_ps[pr][0:64, :],
                        bq_t[0:64, pr : pr + 1],
                        None,
                        ADD,
                    )
                    with nc.allow_low_precision(reason="bf16 qt store"):
                        nc.scalar.activation(
                            qt_r[64:128, 2 * pr + 1, :],
                            qt_ps[pr][64:128, :],
                            Ident,
                            bias=bq_t[64:128, pr : pr + 1],
                        )
                # duplicate halves so score matmuls contract K=128 (2x scores,
                # folded into the exp scale) -- keeps the PE fully row-active.
                for pr in range(NPAIR):
                    nc.sync.dma_start(
                        out=qt_r[64:128, 2 * pr, :], in_=qt_r[0:64, 2 * pr, :]
                    )
                    nc.sync.dma_start(
                        out=qt_r[0:64, 2 * pr + 1, :],
                        in_=qt_r[64:128, 2 * pr + 1, :],
                    )

            # ================= Phase C: attention =================
            # wot is only needed mid-phase-C; its DMA overlaps the start.
            nc.sync.dma_start(out=wot_r[:], in_=wot[:])
            with (
                nc.named_scope("phaseC_attn"),
                tc.tile_pool(name="pp", bufs=4) as pp,
                tc.tile_pool(name="nrm", bufs=2) as nrm,
                tc.tile_pool(name="osb", bufs=3) as osb,
                tc.tile_pool(name="osb1", bufs=5) as osb1,
                tc.tile_pool(name="psS", bufs=2, space="PSUM") as psS,
                tc.tile_pool(name="psAtt", bufs=1, space="PSUM") as psAtt,
                tc.tile_pool(name="psX", bufs=2, space="PSUM") as psX,
            ):
                def emit_rec(hd, den, fill=False):
                    # reciprocal of the softmax denominator on ACT:
                    # rec = exp(-ln(den)); same activation table set as exp.
                    # fill=True: tiny rank-1 matmuls chained off den/ln keep
                    # the PE active through this chain (else HAM re-throttles
                    # after ~3.4us idle and the whole phase-D tail runs at
                    # 1.2 GHz).
                    rec = nrm.tile([1, NSQH, 512], DT_MM, tag="rec",
                                   name=f"rec{hd}")
                    for sh in range(NSQH):
                        t1 = nrm.tile([1, 512], F32, tag="lnd",
                                      name=f"lnd{hd}{sh}")
                        if fill:
                            scr = psX.tile([DH, 512], F32, tag="psX",
                                           name=f"fill{hd}{sh}")
                            nc.tensor.matmul(scr[:], ones_row[0:1, :],
                                             den[0:1, sh, :],
                                             start=True, stop=False)
                            nc.tensor.matmul(scr[:], ones_row[0:1, :],
                                             den[0:1, sh, :],
                                             start=False, stop=True)
                        nc.scalar.activation(t1[:], den[:, sh, :], Ln)
                        if fill:
                            # fp32 rank-1 (4 cyc/row) pinned after ln: dense
                            # PE activity inside the recip chain
                            scr2 = psX.tile([GHEADS, 512], F32, tag="psX",
                                            name=f"fill2{hd}{sh}")
                            nc.tensor.matmul(scr2[:], ones_f[0:1, :],
                                             t1[0:1, :],
                                             start=True, stop=True)
                        with nc.allow_low_precision(reason="softmax recip"):
                            nc.scalar.activation(rec[:, sh, :], t1[:], Exp,
                                                 scale=-1.0)
                    return rec

                def emit_norm(hd, att_sb, rec, sh):
                    pr, h = hd // 2, hd % 2
                    sq_sl = slice(512 * sh, 512 * (sh + 1))
                    rb = psX.tile([DH, 512], F32, tag="psX", name=f"rb{hd}{sh}")
                    nc.tensor.matmul(
                        rb[:], ones_row[0:1, :], rec[0:1, sh, :],
                        start=True, stop=True,
                    )
                    with nc.allow_low_precision(reason="bf16 att store"):
                        nc.vector.tensor_tensor(
                            att_r[64 * h : 64 * (h + 1), pr, sq_sl],
                            att_sb[:, sh, :],
                            rb[:],
                            MUL,
                        )

                def _copy_osb(o_sb, sh, src, eng):
                    dst = o_sb[:, 512 * sh : 512 * (sh + 1)]
                    if eng == "scalar":
                        nc.scalar.activation(
                            dst, src, mybir.ActivationFunctionType.Copy
                        )
                    elif eng == "gpsimd":
                        nc.gpsimd.tensor_copy(dst, src)
                    else:
                        nc.vector.tensor_copy(dst, src)

                def d_jt(jt, pr, pool, tag, shape2, cp_engines):
                    j_sl = slice(128 * jt, 128 * (jt + 1))
                    o_sb = osb.tile([128, SQ], FP16, tag="osb",
                                    name=f"osb{pr}_{jt}")
                    if shape2:
                        o_ps = pool.tile([128, NSQH, 512], F32, tag=tag,
                                         name=f"o{pr}_{jt}")
                        for sh in range(NSQH):
                            nc.tensor.matmul(
                                o_ps[:, sh, :],
                                wot_r[:, pr, j_sl],
                                att_r[:, pr, 512 * sh : 512 * (sh + 1)],
                                start=True,
                                stop=True,
                            )
                        for sh in range(NSQH):
                            _copy_osb(o_sb, sh, o_ps[:, sh, :],
                                      cp_engines[sh % len(cp_engines)])
                    else:
                        for sh in range(NSQH):
                            o_ps = pool.tile([128, 512], F32, tag=tag,
                                             name=f"o{pr}_{jt}_{sh}")
                            nc.tensor.matmul(
                                o_ps[:],
                                wot_r[:, pr, j_sl],
                                att_r[:, pr, 512 * sh : 512 * (sh + 1)],
                                start=True,
                                stop=True,
                            )
                            _copy_osb(o_sb, sh, o_ps[:],
                                      cp_engines[sh % len(cp_engines)])
                    nc.sync.dma_start(out=outp[pr, j_sl, :], in_=o_sb[:])

                # D-pr0 slots: (head, kv) pairs during heads 2-3
                d0_slots = ([(2, kv) for kv in range(6, 32, 2)] +
                            [(2, 31)] +
                            [(3, kv) for kv in range(2, 28, 2)] +
                            [(3, 29)])
                d0_iter = iter(range(NJT))
                d0_map = {slot: jt for slot, jt in zip(d0_slots, d0_iter)}

                pending = []  # (hd, att_sb, den) awaiting norm emission

                for hd in range(GHEADS):
                    att_ps = psAtt.tile([128, SQ], F32, tag="att",
                                        name=f"att{hd}")

                    def pv(kv, p_ap, att_ps=att_ps, hd=hd):
                        for sh in range(NSQH):
                            sq = slice(512 * sh, 512 * (sh + 1))
                            nc.tensor.matmul(
                                att_ps[:, sq],
                                v_r[:, kv, hd, :],
                                p_ap[:, sq],
                                start=(kv == 0),
                                stop=(kv == NKVT - 1),
                            )

                    def extras(hd, kv):
                        # ---- interleaved extra PE work ----
                        if hd == 0:
                            if kv <= 27:
                                v_chunk(kv + 4, psX, "psX")
                            if kv % 4 == 0 and kv <= 24:
                                b = kv // 4 + 1
                                if b + 1 <= NKVB - 1:
                                    kv_dma(b + 1)
                                k_chunk(b, 0, psX, "psX")
                            elif kv % 4 == 1 and kv <= 25:
                                k_chunk(kv // 4 + 1, 1, psX, "psX")
                            elif kv == 26:
                                # batch-duplicate kt halves for heads 1-3
                                nc.gpsimd.dma_start(out=kt_r[0:64, 1, :],
                                                    in_=kt_r[64:128, 1, :])
                                nc.gpsimd.dma_start(out=kt_r[64:128, 2, :],
                                                    in_=kt_r[0:64, 2, :])
                                nc.gpsimd.dma_start(out=kt_r[0:64, 3, :],
                                                    in_=kt_r[64:128, 3, :])
                        if pending and kv in (2, 4):
                            ph, psb, prec = pending[0]
                            emit_norm(ph, psb, prec, 0 if kv == 2 else 1)
                            if kv == 4:
                                pending.pop(0)
                        jt = d0_map.get((hd, kv))
                        if jt is not None:
                            d_jt(jt, 0, psX, "psX", False, ("vector",))

                    pq = []  # pending (kv, p_ap) awaiting PV
                    for kv in range(NKVT):
                        s_ps = psS.tile([128, SQ], F32, tag="s",
                                        name=f"s{hd}_{kv}")
                        kv_sl = slice(128 * kv, 128 * (kv + 1))
                        for sh in range(NSQH):
                            sq = slice(512 * sh, 512 * (sh + 1))
                            nc.tensor.matmul(
                                s_ps[:, sq],
                                kt_r[:, hd, kv_sl],
                                qt_r[:, hd, sq],
                                start=True,
                                stop=True,
                            )
                        p = pp.tile([128, SQ], DT_MM, tag="p",
                                    name=f"p{hd}_{kv}")
                        nc.scalar.activation(p[:], s_ps[:], Exp, scale=0.0625)
                        pq.append((kv, p[:]))
                        if kv >= 1:
                            kv_, p_ = pq.pop(0)
                            pv(kv_, p_)
                            extras(hd, kv_)
                    kv_, p_ = pq.pop(0)
                    pv(kv_, p_)
                    extras(hd, kv_)
                    if hd == GHEADS - 1:
                        d_jt(28, 0, psX, "psX", False, ("vector",))

                    # free att psum banks: copy numerator + denominator to SBUF
                    att_sb = nrm.tile([DH, NSQH, 512], F32, tag="attsb",
                                      name=f"attsb{hd}")
                    den = nrm.tile([1, NSQH, 512], DT_MM, tag="den",
                                   name=f"den{hd}")
                    for sh in range(NSQH):
                        sq_sl = slice(512 * sh, 512 * (sh + 1))
                        nc.vector.tensor_copy(att_sb[:, sh, :],
                                              att_ps[0:DH, sq_sl])
                        nc.vector.tensor_copy(den[:, sh, :],
                                              att_ps[DH : DH + 1, sq_sl])
                    if hd < GHEADS - 1:
                        rec = emit_rec(hd, den)
                        pending.append((hd, att_sb, rec))
                    else:
                        # keep the PE busy through the norm chain (HAM would
                        # re-throttle after ~3.4us idle): held-back D-pr0
                        # tiles act as filler.
                        d_jt(29, 0, psX, "psX", False, ("vector",))
                        rec = emit_rec(hd, den, fill=True)
                        d_jt(30, 0, psX, "psX", False, ("vector",))
                        emit_norm(hd, att_sb, rec, 0)
                        d_jt(31, 0, psX, "psX", False, ("vector",))
                        emit_norm(hd, att_sb, rec, 1)

                # ---- tail: D partial for pair 1 (deep 6-slot pipeline) ----
                for jt in range(NJT):
                    j_sl = slice(128 * jt, 128 * (jt + 1))
                    o_sb = osb1.tile([128, SQ], FP16, tag="osb1",
                                     name=f"osb1_{jt}")
                    m = jt % 4
                    if m == 2:  # psX: two 1-bank half tiles
                        o_half = [
                            psX.tile([128, 512], F32, tag="psX",
                                     name=f"o1_{jt}_{sh}")
                            for sh in range(NSQH)
                        ]
                        parts = [(o_half[0][:], 0), (o_half[1][:], 1)]
                    else:
                        pool = psS if m != 1 else psAtt
                        o_ps = pool.tile([128, SQ], F32,
                                         tag="s" if m != 1 else "att",
                                         name=f"o1_{jt}")
                        parts = [(o_ps[:, 0:512], 0), (o_ps[:, 512:1024], 1)]
                    for ap, sh in parts:
                        nc.tensor.matmul(
                            ap,
                            wot_r[:, 1, j_sl],
                            att_r[:, 1, 512 * sh : 512 * (sh + 1)],
                            start=True,
                            stop=True,
                        )
                    for ap, sh in parts:
                        dst = o_sb[:, 512 * sh : 512 * (sh + 1)]
                        if sh == 0:
                            nc.vector.tensor_copy(dst, ap)
                        else:
                            nc.scalar.activation(
                                dst, ap,
                                mybir.ActivationFunctionType.Copy,
                            )
                    nc.sync.dma_start(out=outp[1, j_sl, :], in_=o_sb[:])

    _NC_CACHE["nc"] = nc
    return nc


def _make_in_maps(inputs):
    ff = np.asarray(inputs["fused_features"], dtype=np.float32)
    kv_in = np.concatenate(
        [
            np.asarray(inputs["text"], dtype=np.float32),
            np.asarray(inputs["image"], dtype=np.float32),
            np.asarray(inputs["audio"], dtype=np.float32),
            np.asarray(inputs["video"], dtype=np.float32),
        ],
        axis=1,
    )
    Wq = np.asarray(inputs["Wq"], dtype=np.float32)
    Wk = np.asarray(inputs["Wk"], dtype=np.float32)
    Wv = np.asarray(inputs["Wv"], dtype=np.float32)
    Wo = np.asarray(inputs["Wo"], dtype=np.float32)
    bq = np.asarray(inputs["bq"], dtype=np.float32)
    bk = np.asarray(inputs["bk"], dtype=np.float32)

    import ml_dtypes

    np_mm = np.dtype(ml_dtypes.bfloat16) if NP_MM == "bfloat16" else np.float32
    ffT = [np.ascontiguousarray(ff[b].T.astype(np_mm)) for b in range(B)]
    kvT = [
        np.ascontiguousarray(
            kv_in[b].T.astype(np_mm).reshape(HID, NKVB, 512).transpose(1, 0, 2)
        )
        for b in range(B)
    ]
    WqT = np.ascontiguousarray(Wq.T.astype(np_mm))  # [4096, 1024]
    WkT = np.ascontiguousarray(Wk.T.astype(np_mm))  # [1024, 1024]
    WvT = np.ascontiguousarray(Wv.T.astype(np_mm))
    WoT = np.ascontiguousarray(Wo.T.astype(np_mm))  # [1024, 4096]

    # pre-tile to [128 partitions, ...] p-major contiguous layouts
    fftP = [
        np.ascontiguousarray(
            f.reshape(8, 4, 128, SQ).transpose(2, 0, 1, 3)
        )  # [p, chunk, e, s]
        for f in ffT
    ]
    kvP = [
        np.ascontiguousarray(
            k.reshape(NKVB, NFT_KV, 128, 512).transpose(2, 0, 1, 3)
        )  # [p, kb, ft, n]
        for k in kvT
    ]

    in_maps = []
    for c in range(NCORES):
        b, hg = divmod(c, HG)
        fs = slice(GF * hg, GF * (hg + 1))
        wq = np.ascontiguousarray(
            WqT[:, fs].reshape(NFT_Q, 128, NPAIR, 128).transpose(1, 0, 2, 3)
        )  # [p, kt, pr, d]
        wk = np.ascontiguousarray(
            WkT[:, fs].reshape(NFT_KV, 128, NPAIR, 128).transpose(1, 0, 2, 3)
        )  # [p, ft, pr, d]
        wv = np.ascontiguousarray(
            WvT[:, fs].reshape(NFT_KV, 128, GF).transpose(1, 0, 2)
        )  # [p, ft, d]
        wo = np.ascontiguousarray(
            WoT[fs, :].reshape(NPAIR, 128, DOUT).transpose(1, 0, 2)
        )  # [p, pr, j]
        in_maps.append(
            {
                "fft": fftP[b],
                "kvt": kvP[b],
                "wqt": wq,
                "wkt": wk,
                "wvt": wv,
                "wot": wo,
                "bq": np.ascontiguousarray(bq[fs].reshape(NPAIR, 128).T),
                "bk": np.ascontiguousarray(bk[fs].reshape(NPAIR, 128).T),
            }
        )
    return in_maps


def _assemble(results, bo, Wo, bv):
    out = np.zeros((B, SQ, DOUT), dtype=np.float32)
    for c in range(NCORES):
        b = c // HG
        partial = results[c]["outp"].astype(np.float32)  # [NPAIR, DOUT, SQ]
        out[b] += (partial[0] + partial[1]).T
    bo_prime = np.asarray(bo, dtype=np.float32) + np.asarray(
        Wo, dtype=np.float32
    ) @ np.asarray(bv, dtype=np.float32)
    out += bo_prime
    return out


def run_spmd(inputs, trace=False):
    nc = build()
    in_maps = _make_in_maps(inputs)
    r = run_bass_kernel_spmd(nc, in_maps, list(range(NCORES)), trace=trace)
    return _assemble(r.results, inputs["bo"], inputs["Wo"], inputs["bv"]), r


def kernel(**inputs) -> np.ndarray:
    out, _ = run_spmd(inputs, trace=False)
    return out
